# revision 8
# baseline (speedup 1.0000x reference)
"""FPN ROI-align (crop + bilinear + 2x2 maxpool) Trainium2 kernel.

Problem: p2..p5 FPN pyramid [1,256,S,S] (S=256,128,64,32), rois [1000,5]
-> out [1000, 256, 7, 7] float32.

Strategy (8 NeuronCores, SPMD):
  - Shard ROIs: 125 per core; replicate the features.
  - Host builds a 4-corner NHWC table T4: row (lvl,y,x) =
    [T(y,x) | T(y+1c,x) | T(y,x+1c) | T(y+1c,x+1c)] (edge-clamped),
    1024 f16 per row.  One gathered row = all four bilinear corners of
    one sample: [A|B | C|D] (left column pair | right column pair).
  - Gathers use the Q7 ext-isa dma_gather (mlp library): ONE instruction
    fetches 512+ rows (int16 indices, ~1us fixed SWDGE cost amortized).
    int16 range forces segmenting T4 into four <=32768-row windows
    (p2 top half / p2 bottom half / p2 straddle window / p3+p4+p5);
    each output point is routed to a window that contains all four of
    its pool-plane sample rows.  Pad slots use idx 0 with zero weights.
  - Separable bilinear on-chip: xb = wx0*[A|B] + wx1*[C|D] (512-wide),
    o = wt*xb_top + wb*xb_bot (256-wide) via tensor_scalar +
    scalar_tensor_tensor (4x DVE perf mode on packed f16).  ACT engine
    takes the scalar multiplies for planes 0/1.
  - 2x2 maxpool = elementwise max across the four planes (chunk-wide).
  - Host un-permutes gather positions back to (roi, oy, ox) order.
"""
import os
import sys

import numpy as np

for _p in ("/opt/trn_rl_repo", "/root/.axon_site/_ro/trn_rl_repo"):
    if _p not in sys.path and os.path.isdir(_p):
        sys.path.append(_p)

import bass_rust  # noqa: E402
from concourse import bass, mybir  # noqa: E402
import concourse.tile as tile  # noqa: E402
from concourse.bass_utils import run_bass_kernel_spmd  # noqa: E402
from concourse.library_config import mlp  # noqa: E402
from concourse.library_overlay import lower_extended_insts  # noqa: E402
from concourse.vector_clock import ScopedClock  # noqa: E402

_MAX_WAITS = 1
_NOP_SEQ = [0]


def _patched_add_instruction(self, inst):
    """Wrap TileContext._add_instruction: the pinned walrus codegen allows
    at most one sync wait per instruction, so hoist excess waits onto
    single-wait NOPs queued just before on the same engine."""
    si = inst.sync_info
    if si is not None and len(si.on_wait) > _MAX_WAITS:
        waits = list(si.on_wait)
        extra, keep = waits[:-_MAX_WAITS], waits[-_MAX_WAITS:]
        for w in extra:
            _NOP_SEQ[0] += 1
            nop = bass_rust.InstNoOp(name=f"wsplit-{_NOP_SEQ[0]}", engine=inst.engine)
            nop.sync_info = bass_rust.SyncInfo(on_wait=[w], on_update=[])
            nop.bass_nofuse = True
            _orig_add_instruction(self, nop)
        inst.sync_info = bass_rust.SyncInfo(
            on_wait=keep, on_update=list(si.on_update)
        )
    _orig_add_instruction(self, inst)


_orig_add_instruction = tile.TileContext._add_instruction
if getattr(tile.TileContext, "_wsplit_patched", False):
    _orig_add_instruction = tile.TileContext._wsplit_orig
tile.TileContext._add_instruction = _patched_add_instruction
tile.TileContext._wsplit_patched = True
tile.TileContext._wsplit_orig = _orig_add_instruction


def _split_wait_drain_and_barrier(self, tick_clock, wait_clock):
    """Replacement for TileContext._drain_and_barrier (same wait limit)."""
    nc = self.nc
    probe = nc.sync.nop(nofuse=True)
    wait_clock.add_sem_waits(
        probe.ins, ScopedClock({None: tick_clock.global_clock})
    )
    si = probe.ins.sync_info
    waits = list(si.on_wait) if si is not None else []
    if si is not None:
        probe.ins.sync_info = bass_rust.SyncInfo(on_wait=waits[:1], on_update=[])
    for w in waits[1:]:
        n = nc.sync.nop(nofuse=True)
        n.ins.sync_info = bass_rust.SyncInfo(on_wait=[w], on_update=[])
    nc.sync.drain()

    nc.all_engine_barrier()
    assert self.sems is not None
    popped = nc._tile_sem_poison_stack.pop()
    assert popped is self._sem_poison
    nc.clear_and_free_semaphores(list(self.sems.allocated().values()))
    nc.all_engine_barrier()


tile.TileContext._drain_and_barrier = _split_wait_drain_and_barrier

# ---------------------------------------------------------------- constants
POOL = 7
PRE = 14
C = 256
N_ROIS = 1000
N_CORES = 8
ROIS_PER_CORE = N_ROIS // N_CORES          # 125
NPTS = ROIS_PER_CORE * POOL * POOL         # 6125 output points per core
LEVEL_HW = np.array([256, 128, 64, 32], np.int64)
BASES = np.array([0, 65536, 81920, 86016], np.int64)
R_TAB = 87040

# T4 row-index windows (start, row_count); every point's 4 sample rows
# must fit one window, indexed locally with int16 (< 32768).
SEGS = [
    (0, 32768),        # p2, y in [0, 128)
    (32768, 32768),    # p2, y in [128, 256)
    (16384, 32768),    # p2, y in [64, 192) -- boundary straddlers
    (65536, 21504),    # p3 + p4 + p5
]
CHUNK = 512                                # points per dma_gather

import ml_dtypes  # noqa: E402

TABLE_NP_DT = ml_dtypes.bfloat16
TABLE_MB_DT = mybir.dt.bfloat16
BLEND_MB_DT = mybir.dt.bfloat16

# ---------------------------------------------------------------- host math


def _build_t4(p2, p3, p4, p5):
    """4-corner table [R_TAB, 1024]: row (lvl,y,x) =
    [T(y,x) | T(y+1c,x) | T(y,x+1c) | T(y+1c,x+1c)]."""
    parts = []
    for p in (p2, p3, p4, p5):
        L = np.transpose(p[0], (1, 2, 0)).astype(TABLE_NP_DT)  # [H, W, C]
        H, W = L.shape[0], L.shape[1]
        yb = np.minimum(np.arange(H) + 1, H - 1)
        xr = np.minimum(np.arange(W) + 1, W - 1)
        A = L
        B = L[yb]
        Cc = L[:, xr]
        D = L[yb][:, xr]
        parts.append(
            np.concatenate([A, B, Cc, D], axis=-1).reshape(-1, 4 * C)
        )
    return np.ascontiguousarray(np.concatenate(parts, axis=0))


def _roi_sample_data(rois):
    """f32-faithful mirror of the reference's coordinate math.

    Returns ilo [N,14,14] int64 (T4 row) and separable weights
    cw0, cw1 [N,14] (x, bounds-masked+edge-folded) and w_top, w_bot
    [N,14] (y, degenerate rows folded onto top)."""
    f32 = np.float32
    x1 = rois[:, 1].astype(f32)
    y1 = rois[:, 2].astype(f32)
    x2 = rois[:, 3].astype(f32)
    y2 = rois[:, 4].astype(f32)
    w = np.where(x2 - x1 <= 0, f32(1e-14), x2 - x1).astype(f32)
    h = np.where(y2 - y1 <= 0, f32(1e-14), y2 - y1).astype(f32)
    kf = f32(4.0) + np.log2(np.sqrt(w * h) / f32(224.0)).astype(f32)
    kf = np.clip(kf, f32(2.0), f32(5.0))
    k = np.round(kf)
    scale = np.exp2(k).astype(f32)
    lvl = k.astype(np.int64) - 2
    Wl = LEVEL_HW[lvl]
    x1s, y1s, x2s, y2s = x1 / scale, y1 / scale, x2 / scale, y2 / scale

    t = np.linspace(-1.0, 1.0, PRE, dtype=f32)
    px = (x1s + x2s)[:, None] * f32(0.5) + t[None, :] * ((x2s - x1s)[:, None] * f32(0.5))
    py = (y1s + y2s)[:, None] * f32(0.5) + t[None, :] * ((y2s - y1s)[:, None] * f32(0.5))

    u0 = np.floor(px)
    dx = (px - u0).astype(f32)
    u0i = u0.astype(np.int64)
    in_u0 = (u0i >= 0) & (u0i < Wl[:, None])
    in_u1 = (u0i + 1 >= 0) & (u0i + 1 < Wl[:, None])
    a0 = (f32(1.0) - dx) * in_u0
    a1 = dx * in_u1
    bx = np.clip(u0i, 0, (Wl - 2)[:, None])
    cw0 = a0 * (u0i == bx) + a1 * (u0i + 1 == bx)
    cw1 = a0 * (u0i == bx + 1) + a1 * (u0i + 1 == bx + 1)

    v0 = np.floor(py)
    dy = (py - v0).astype(f32)
    v0i = v0.astype(np.int64)
    b0 = (f32(1.0) - dy) * ((v0i >= 0) & (v0i < Wl[:, None]))
    b1 = dy * ((v0i + 1 >= 0) & (v0i + 1 < Wl[:, None]))
    ylo = np.clip(v0i, 0, (Wl - 1)[:, None])
    yhi = np.clip(v0i + 1, 0, (Wl - 1)[:, None])
    same = yhi == ylo
    w_top = b0 + b1 * same
    w_bot = b1 * (~same)

    base = BASES[lvl]
    ilo = base[:, None, None] + ylo[:, :, None] * Wl[:, None, None] + bx[:, None, :]
    return (
        ilo,
        cw0.astype(f32),
        cw1.astype(f32),
        w_top.astype(f32),
        w_bot.astype(f32),
    )


def _route_core(rois_chunk):
    """Segment-route one core's output points.

    Returns (seg_pids, rows, weights) where seg_pids[s] is the pid list
    (pid = roi*49 + oy*7 + ox) routed to segment s, rows [4, NPTS] is
    the global T4 row per plane, and weights [4, NPTS, 4] holds
    (wx0, wx1, wt, wb) per plane."""
    N = rois_chunk.shape[0]
    ilo, cw0, cw1, w_top, w_bot = _roi_sample_data(rois_chunk)

    oy, ox = np.meshgrid(np.arange(POOL), np.arange(POOL), indexing="ij")
    oyf = oy.reshape(-1)
    oxf = ox.reshape(-1)
    nn = np.repeat(np.arange(N), 49)
    oyr = np.tile(oyf, N)
    oxr = np.tile(oxf, N)

    rows = np.zeros((4, NPTS), np.int64)
    wts = np.zeros((4, NPTS, 4), np.float32)
    for q in range(4):
        a, b = q // 2, q % 2
        iy = 2 * oyr + a
        ix = 2 * oxr + b
        rows[q] = ilo[nn, iy, ix]
        # folded corner weights for chunks [A|B|C|D]
        wts[q, :, 0] = w_top[nn, iy] * cw0[nn, ix]
        wts[q, :, 1] = w_bot[nn, iy] * cw0[nn, ix]
        wts[q, :, 2] = w_top[nn, iy] * cw1[nn, ix]
        wts[q, :, 3] = w_bot[nn, iy] * cw1[nn, ix]

    rmin = rows.min(axis=0)
    rmax = rows.max(axis=0)
    seg_pids = []
    assigned = np.zeros(NPTS, bool)
    for s, (base, cnt) in enumerate(SEGS):
        ok = (~assigned) & (rmin >= base) & (rmax < base + cnt)
        seg_pids.append(np.nonzero(ok)[0])
        assigned |= ok
    assert assigned.all(), "point not routable to any T4 segment"
    return seg_pids, rows, wts


def _chunks_for(cap):
    """Split a segment capacity (multiple of 128) into dma_gather chunks."""
    out = []
    left = cap
    while left > 0:
        c = min(CHUNK, left)
        out.append(c)
        left -= c
    return out


def _pack_core(seg_pids, rows, wts, caps):
    """Pack one core's gather indices / weights / output permutation.

    Returns idx16 [4, 128, NIT//16] int16 (dma_gather wrapped layout,
    replicated across the 8 Q7 core stripes), wtsp [4, 128, NBT*4] f32,
    perm [NIT] int64 (pid per position, -1 for pad)."""
    NIT = sum(caps)
    NBT = NIT // 128
    idx16 = np.zeros((4, 128, NIT // 16), np.int16)
    wtsp = np.zeros((4, 128, NBT * 4), np.float32)
    perm = np.full(NIT, -1, np.int64)

    pos0 = 0
    for s, (base, cnt) in enumerate(SEGS):
        pids = seg_pids[s]
        n = len(pids)
        assert n <= caps[s]
        # positions pos0..pos0+caps[s]; chunk-wrapped idx layout
        perm[pos0 : pos0 + n] = pids
        for q in range(4):
            loc = np.zeros(caps[s], np.int64)
            loc[:n] = rows[q][pids] - base
            assert loc.max(initial=0) < cnt
            # weights land at (partition, batch) of the global position
            gpos = pos0 + np.arange(n)
            pq = gpos % 128
            bq = gpos // 128
            for kk in range(4):
                wtsp[q, pq, bq * 4 + kk] = wts[q, pids, kk]
            # idx wrapped per chunk: position i in chunk -> [i%16, i//16]
            coff = pos0
            for ck in _chunks_for(caps[s]):
                li = np.arange(ck)
                v = loc[coff - pos0 : coff - pos0 + ck]
                idx16[q, li % 16, (coff + li * 0) // 16 + li // 16] = v.astype(
                    np.int16
                )
                coff += ck
        pos0 += caps[s]

    for rep in range(1, 8):
        idx16[:, rep * 16 : (rep + 1) * 16, :] = idx16[:, 0:16, :]
    return idx16, wtsp, perm


# ---------------------------------------------------------------- device program

_NC_CACHE = {}


def build_program(caps):
    key = tuple(caps)
    if key in _NC_CACHE:
        return _NC_CACHE[key]
    NIT = sum(caps)
    NBT = NIT // 128
    f32 = mybir.dt.float32
    fb = BLEND_MB_DT
    nc = bass.Bass(num_swdge_queues=4)
    t4 = nc.declare_dram_parameter("t4", [R_TAB, 4 * C], TABLE_MB_DT, isOutput=False)
    idx_p = nc.declare_dram_parameter(
        "idx", [4, 128, NIT // 16], mybir.dt.int16, isOutput=False
    )
    wts_p = nc.declare_dram_parameter("wts", [4, 128, NBT * 4], f32, isOutput=False)
    out_p = nc.declare_dram_parameter("out", [128, NBT * C], fb, isOutput=True)

    Copy = mybir.ActivationFunctionType.Copy
    add = mybir.AluOpType.add
    mult = mybir.AluOpType.mult
    amax = mybir.AluOpType.max

    # (seg, chunk_points, col16_offset, batch_offset) schedule
    sched = []
    col16 = 0
    batch = 0
    for s, cap in enumerate(caps):
        for ck in _chunks_for(cap):
            sched.append((s, ck, col16, batch))
            col16 += ck // 16
            batch += ck // 128

    with tile.TileContext(nc) as tc:
        with (
            tc.tile_pool(name="const", bufs=1) as cpool,
            tc.tile_pool(name="gp", bufs=8) as gpool,
            tc.tile_pool(name="tp", bufs=12) as tpool,
            tc.tile_pool(name="op", bufs=8) as opool,
            tc.tile_pool(name="mp", bufs=4) as mpool,
            tc.tile_pool(name="fp", bufs=3) as fpool,
        ):
            nc.gpsimd.load_library(mlp)
            _regs = {}

            def nreg(v):
                if v not in _regs:
                    r = nc.gpsimd.register(f"ni{v}").__enter__()
                    nc.gpsimd.reg_mov(r, v)
                    _regs[v] = r
                return _regs[v]

            idx_t = []
            wts_t = []
            for q in range(4):
                it = cpool.tile([128, NIT // 16], mybir.dt.int16, tag=f"idx{q}")
                wt = cpool.tile([128, NBT * 4], f32, tag=f"wts{q}")
                nc.sync.dma_start(out=it[:], in_=idx_p[q])
                nc.sync.dma_start(out=wt[:], in_=wts_p[q])
                idx_t.append(it)
                wts_t.append(wt)

            gi = [0]
            for s, ck, c16, b0 in sched:
                base, cnt = SEGS[s]
                nb = ck // 128
                otiles = []
                for q in range(4):
                    gt = gpool.tile([128, nb * 4 * C], TABLE_MB_DT, tag="g")
                    nc.gpsimd.dma_gather(
                        out_ap=gt[:].rearrange("p (n e) -> p n e", e=4 * C),
                        in_ap=t4[base : base + cnt],
                        idxs_ap=idx_t[q][:, c16 : c16 + ck // 16],
                        num_idxs=ck,
                        num_idxs_reg=nreg(ck),
                        elem_size=4 * C,
                        queue_num=gi[0] % 4,
                    )
                    gi[0] += 1
                    ot = opool.tile([128, nb * C], fb, tag="o")
                    for bi in range(nb):
                        bb = b0 + bi
                        wcol = bb * 4
                        wv = lambda kk: wts_t[q][:, wcol + kk : wcol + kk + 1]
                        ch = lambda kk: gt[
                            :, bi * 1024 + kk * 256 : bi * 1024 + (kk + 1) * 256
                        ]
                        t1 = tpool.tile([128, 256], fb, tag="t1")
                        t2 = tpool.tile([128, 256], fb, tag="t2")
                        t3 = tpool.tile([128, 256], fb, tag="t3")
                        # folded blend: o = WA*A + WB*B + WC*C + WD*D
                        if q < 2:
                            nc.scalar.activation(t1[:], ch(0), Copy, scale=wv(0))
                        else:
                            nc.vector.tensor_scalar(t1[:], ch(0), wv(0), None, mult)
                        nc.vector.scalar_tensor_tensor(
                            t2[:], ch(1), wv(1), t1[:], mult, add
                        )
                        nc.vector.scalar_tensor_tensor(
                            t3[:], ch(2), wv(2), t2[:], mult, add
                        )
                        nc.vector.scalar_tensor_tensor(
                            ot[:, bi * C : (bi + 1) * C],
                            ch(3), wv(3), t3[:], mult, add,
                        )
                    otiles.append(ot)

                m01 = mpool.tile([128, nb * C], fb, tag="m")
                m23 = mpool.tile([128, nb * C], fb, tag="m")
                fo = fpool.tile([128, nb * C], fb, tag="f")
                nc.vector.tensor_tensor(m01[:], otiles[0][:], otiles[1][:], amax)
                nc.vector.tensor_tensor(m23[:], otiles[2][:], otiles[3][:], amax)
                nc.vector.tensor_tensor(fo[:], m01[:], m23[:], amax)
                nc.sync.dma_start(
                    out=out_p[:, b0 * C : (b0 + nb) * C], in_=fo[:]
                )

    lower_extended_insts(nc)
    _NC_CACHE[key] = nc
    return nc


# ---------------------------------------------------------------- entry point


def kernel(p2, p3, p4, p5, rois, **run_kwargs):
    p2, p3, p4, p5, rois = (
        np.asarray(p2), np.asarray(p3), np.asarray(p4), np.asarray(p5),
        np.asarray(rois),
    )
    t4 = _build_t4(p2, p3, p4, p5)

    routed = []
    counts = np.zeros((N_CORES, len(SEGS)), np.int64)
    for core in range(N_CORES):
        chunk = rois[core * ROIS_PER_CORE : (core + 1) * ROIS_PER_CORE]
        seg_pids, rows, wts = _route_core(chunk)
        routed.append((seg_pids, rows, wts))
        counts[core] = [len(p) for p in seg_pids]

    caps = [int(-(-counts[:, s].max() // 128) * 128) for s in range(len(SEGS))]
    nc = build_program(caps)

    in_maps = []
    perms = []
    for core in range(N_CORES):
        seg_pids, rows, wts = routed[core]
        idx16, wtsp, perm = _pack_core(seg_pids, rows, wts, caps)
        in_maps.append({"t4": t4, "idx": idx16, "wts": wtsp})
        perms.append(perm)
    res = run_bass_kernel_spmd(nc, in_maps, core_ids=list(range(N_CORES)), **run_kwargs)

    NIT = sum(caps)
    NBT = NIT // 128
    outs = []
    for core in range(N_CORES):
        flat = (
            np.asarray(res.results[core]["out"])
            .astype(np.float32)
            .reshape(128, NBT, C)
            .transpose(1, 0, 2)
            .reshape(NIT, C)
        )
        perm = perms[core]
        pts = np.zeros((NPTS, C), np.float32)
        valid = perm >= 0
        pts[perm[valid]] = flat[valid]
        outs.append(
            pts.reshape(ROIS_PER_CORE, POOL, POOL, C).transpose(0, 3, 1, 2)
        )
    out = np.ascontiguousarray(np.concatenate(outs, axis=0))
    if run_kwargs:
        return out, res
    return out


# revision 19
# speedup vs baseline: 1.1877x; 1.1877x over previous
"""FPN ROI-align (crop + bilinear + 2x2 maxpool) Trainium2 kernel.

Problem: p2..p5 FPN pyramid [1,256,S,S] (S=256,128,64,32), rois [1000,5]
-> out [1000, 256, 7, 7] float32.

Strategy (8 NeuronCores, SPMD):
  - Shard ROIs: 125 per core; replicate the features.
  - Host builds a 4-corner NHWC table T4: row (lvl,y,x) =
    [T(y,x) | T(y+1c,x) | T(y,x+1c) | T(y+1c,x+1c)] (edge-clamped),
    1024 f16 per row.  One gathered row = all four bilinear corners of
    one sample: [A|B | C|D] (left column pair | right column pair).
  - Gathers use the Q7 ext-isa dma_gather (mlp library): ONE instruction
    fetches 512+ rows (int16 indices, ~1us fixed SWDGE cost amortized).
    int16 range forces segmenting T4 into four <=32768-row windows
    (p2 top half / p2 bottom half / p2 straddle window / p3+p4+p5);
    each output point is routed to a window that contains all four of
    its pool-plane sample rows.  Pad slots use idx 0 with zero weights.
  - Separable bilinear on-chip: xb = wx0*[A|B] + wx1*[C|D] (512-wide),
    o = wt*xb_top + wb*xb_bot (256-wide) via tensor_scalar +
    scalar_tensor_tensor (4x DVE perf mode on packed f16).  ACT engine
    takes the scalar multiplies for planes 0/1.
  - 2x2 maxpool = elementwise max across the four planes (chunk-wide).
  - Host un-permutes gather positions back to (roi, oy, ox) order.
"""
import os
import sys

import numpy as np

for _p in ("/opt/trn_rl_repo", "/root/.axon_site/_ro/trn_rl_repo"):
    if _p not in sys.path and os.path.isdir(_p):
        sys.path.append(_p)

import bass_rust  # noqa: E402
from concourse import bass, mybir  # noqa: E402
import concourse.tile as tile  # noqa: E402
from concourse.bass_utils import run_bass_kernel_spmd  # noqa: E402
from concourse.library_config import mlp  # noqa: E402
from concourse.library_overlay import lower_extended_insts  # noqa: E402
from concourse.vector_clock import ScopedClock  # noqa: E402

_MAX_WAITS = 1
_NOP_SEQ = [0]


def _patched_add_instruction(self, inst):
    """Wrap TileContext._add_instruction: the pinned walrus codegen allows
    at most one sync wait per instruction, so hoist excess waits onto
    single-wait NOPs queued just before on the same engine."""
    si = inst.sync_info
    if si is not None and len(si.on_wait) > _MAX_WAITS:
        waits = list(si.on_wait)
        extra, keep = waits[:-_MAX_WAITS], waits[-_MAX_WAITS:]
        for w in extra:
            _NOP_SEQ[0] += 1
            nop = bass_rust.InstNoOp(name=f"wsplit-{_NOP_SEQ[0]}", engine=inst.engine)
            nop.sync_info = bass_rust.SyncInfo(on_wait=[w], on_update=[])
            nop.bass_nofuse = True
            _orig_add_instruction(self, nop)
        inst.sync_info = bass_rust.SyncInfo(
            on_wait=keep, on_update=list(si.on_update)
        )
    _orig_add_instruction(self, inst)


_orig_add_instruction = tile.TileContext._add_instruction
if getattr(tile.TileContext, "_wsplit_patched", False):
    _orig_add_instruction = tile.TileContext._wsplit_orig
tile.TileContext._add_instruction = _patched_add_instruction
tile.TileContext._wsplit_patched = True
tile.TileContext._wsplit_orig = _orig_add_instruction


def _split_wait_drain_and_barrier(self, tick_clock, wait_clock):
    """Replacement for TileContext._drain_and_barrier (same wait limit)."""
    nc = self.nc
    probe = nc.sync.nop(nofuse=True)
    wait_clock.add_sem_waits(
        probe.ins, ScopedClock({None: tick_clock.global_clock})
    )
    si = probe.ins.sync_info
    waits = list(si.on_wait) if si is not None else []
    if si is not None:
        probe.ins.sync_info = bass_rust.SyncInfo(on_wait=waits[:1], on_update=[])
    for w in waits[1:]:
        n = nc.sync.nop(nofuse=True)
        n.ins.sync_info = bass_rust.SyncInfo(on_wait=[w], on_update=[])
    nc.sync.drain()

    nc.all_engine_barrier()
    assert self.sems is not None
    popped = nc._tile_sem_poison_stack.pop()
    assert popped is self._sem_poison
    nc.clear_and_free_semaphores(list(self.sems.allocated().values()))
    nc.all_engine_barrier()


tile.TileContext._drain_and_barrier = _split_wait_drain_and_barrier

# ---------------------------------------------------------------- constants
POOL = 7
PRE = 14
C = 256
N_ROIS = 1000
N_CORES = 8
ROIS_PER_CORE = N_ROIS // N_CORES          # 125
NPTS = ROIS_PER_CORE * POOL * POOL         # 6125 output points per core
LEVEL_HW = np.array([256, 128, 64, 32], np.int64)
BASES = np.array([0, 65536, 81920, 86016], np.int64)
R_TAB = 87040

# T4 row-index windows (start, row_count); every point's 4 sample rows
# must fit one window, indexed locally with int16 (< 32768).
SEGS = [
    (0, 32768),        # p2, y in [0, 128)
    (32768, 32768),    # p2, y in [128, 256)
    (16384, 32768),    # p2, y in [64, 192) -- boundary straddlers
    (65536, 21504),    # p3 + p4 + p5
]
CHUNK = 512                                # points per dma_gather

import ml_dtypes  # noqa: E402

TABLE_NP_DT = ml_dtypes.bfloat16
TABLE_MB_DT = mybir.dt.bfloat16
BLEND_MB_DT = mybir.dt.bfloat16

# ---------------------------------------------------------------- host math


def _build_t4(p2, p3, p4, p5):
    """4-corner table [R_TAB, 1024]: row (lvl,y,x) =
    [T(y,x) | T(y+1c,x) | T(y,x+1c) | T(y+1c,x+1c)]."""
    parts = []
    for p in (p2, p3, p4, p5):
        L = np.transpose(p[0], (1, 2, 0)).astype(TABLE_NP_DT)  # [H, W, C]
        H, W = L.shape[0], L.shape[1]
        yb = np.minimum(np.arange(H) + 1, H - 1)
        xr = np.minimum(np.arange(W) + 1, W - 1)
        A = L
        B = L[yb]
        Cc = L[:, xr]
        D = L[yb][:, xr]
        parts.append(
            np.concatenate([A, B, Cc, D], axis=-1).reshape(-1, 4 * C)
        )
    return np.ascontiguousarray(np.concatenate(parts, axis=0))


def _roi_sample_data(rois):
    """f32-faithful mirror of the reference's coordinate math.

    Returns ilo [N,14,14] int64 (T4 row) and separable weights
    cw0, cw1 [N,14] (x, bounds-masked+edge-folded) and w_top, w_bot
    [N,14] (y, degenerate rows folded onto top)."""
    f32 = np.float32
    x1 = rois[:, 1].astype(f32)
    y1 = rois[:, 2].astype(f32)
    x2 = rois[:, 3].astype(f32)
    y2 = rois[:, 4].astype(f32)
    w = np.where(x2 - x1 <= 0, f32(1e-14), x2 - x1).astype(f32)
    h = np.where(y2 - y1 <= 0, f32(1e-14), y2 - y1).astype(f32)
    kf = f32(4.0) + np.log2(np.sqrt(w * h) / f32(224.0)).astype(f32)
    kf = np.clip(kf, f32(2.0), f32(5.0))
    k = np.round(kf)
    scale = np.exp2(k).astype(f32)
    lvl = k.astype(np.int64) - 2
    Wl = LEVEL_HW[lvl]
    x1s, y1s, x2s, y2s = x1 / scale, y1 / scale, x2 / scale, y2 / scale

    t = np.linspace(-1.0, 1.0, PRE, dtype=f32)
    px = (x1s + x2s)[:, None] * f32(0.5) + t[None, :] * ((x2s - x1s)[:, None] * f32(0.5))
    py = (y1s + y2s)[:, None] * f32(0.5) + t[None, :] * ((y2s - y1s)[:, None] * f32(0.5))

    u0 = np.floor(px)
    dx = (px - u0).astype(f32)
    u0i = u0.astype(np.int64)
    in_u0 = (u0i >= 0) & (u0i < Wl[:, None])
    in_u1 = (u0i + 1 >= 0) & (u0i + 1 < Wl[:, None])
    a0 = (f32(1.0) - dx) * in_u0
    a1 = dx * in_u1
    bx = np.clip(u0i, 0, (Wl - 2)[:, None])
    cw0 = a0 * (u0i == bx) + a1 * (u0i + 1 == bx)
    cw1 = a0 * (u0i == bx + 1) + a1 * (u0i + 1 == bx + 1)

    v0 = np.floor(py)
    dy = (py - v0).astype(f32)
    v0i = v0.astype(np.int64)
    b0 = (f32(1.0) - dy) * ((v0i >= 0) & (v0i < Wl[:, None]))
    b1 = dy * ((v0i + 1 >= 0) & (v0i + 1 < Wl[:, None]))
    ylo = np.clip(v0i, 0, (Wl - 1)[:, None])
    yhi = np.clip(v0i + 1, 0, (Wl - 1)[:, None])
    same = yhi == ylo
    w_top = b0 + b1 * same
    w_bot = b1 * (~same)

    base = BASES[lvl]
    ilo = base[:, None, None] + ylo[:, :, None] * Wl[:, None, None] + bx[:, None, :]
    return (
        ilo,
        cw0.astype(f32),
        cw1.astype(f32),
        w_top.astype(f32),
        w_bot.astype(f32),
    )


def _route_core(rois_chunk):
    """Segment-route one core's output points.

    Returns (seg_pids, rows, weights) where seg_pids[s] is the pid list
    (pid = roi*49 + oy*7 + ox) routed to segment s, rows [4, NPTS] is
    the global T4 row per plane, and weights [4, NPTS, 4] holds
    (wx0, wx1, wt, wb) per plane."""
    N = rois_chunk.shape[0]
    ilo, cw0, cw1, w_top, w_bot = _roi_sample_data(rois_chunk)

    oy, ox = np.meshgrid(np.arange(POOL), np.arange(POOL), indexing="ij")
    oyf = oy.reshape(-1)
    oxf = ox.reshape(-1)
    nn = np.repeat(np.arange(N), 49)
    oyr = np.tile(oyf, N)
    oxr = np.tile(oxf, N)

    rows = np.zeros((4, NPTS), np.int64)
    wts = np.zeros((4, NPTS, 4), np.float32)
    for q in range(4):
        a, b = q // 2, q % 2
        iy = 2 * oyr + a
        ix = 2 * oxr + b
        rows[q] = ilo[nn, iy, ix]
        # folded corner weights for chunks [A|B|C|D]
        wts[q, :, 0] = w_top[nn, iy] * cw0[nn, ix]
        wts[q, :, 1] = w_bot[nn, iy] * cw0[nn, ix]
        wts[q, :, 2] = w_top[nn, iy] * cw1[nn, ix]
        wts[q, :, 3] = w_bot[nn, iy] * cw1[nn, ix]

    rmin = rows.min(axis=0)
    rmax = rows.max(axis=0)
    seg_pids = []
    assigned = np.zeros(NPTS, bool)
    for s, (base, cnt) in enumerate(SEGS):
        ok = (~assigned) & (rmin >= base) & (rmax < base + cnt)
        seg_pids.append(np.nonzero(ok)[0])
        assigned |= ok
    assert assigned.all(), "point not routable to any T4 segment"
    return seg_pids, rows, wts


def _chunks_for(cap):
    """Split a segment capacity (multiple of 128) into dma_gather chunks."""
    out = []
    left = cap
    while left > 0:
        c = min(CHUNK, left)
        out.append(c)
        left -= c
    return out


def _pack_core(seg_pids, rows, wts, caps):
    """Pack one core's gather indices / weights / output permutation.

    Returns idx16 [4, 128, NIT//16] int16 (dma_gather wrapped layout,
    replicated across the 8 Q7 core stripes), wtsp [4, 128, NBT*4] f32,
    perm [NIT] int64 (pid per position, -1 for pad)."""
    NIT = sum(caps)
    NBT = NIT // 128
    idx16 = np.zeros((4, 128, NIT // 16), np.int16)
    wtsp = np.zeros((4, 128, NBT * 4), TABLE_NP_DT)
    wtsa = np.zeros((4, 128, NBT), np.float32)
    perm = np.full(NIT, -1, np.int64)

    pos0 = 0
    for s, (base, cnt) in enumerate(SEGS):
        pids = seg_pids[s]
        n = len(pids)
        assert n <= caps[s]
        # positions pos0..pos0+caps[s]; chunk-wrapped idx layout
        perm[pos0 : pos0 + n] = pids
        for q in range(4):
            loc = np.zeros(caps[s], np.int64)
            loc[:n] = rows[q][pids] - base
            assert loc.max(initial=0) < cnt
            # weights land at (partition, batch) of the global position
            gpos = pos0 + np.arange(n)
            pq = gpos % 128
            bq = gpos // 128
            for kk in range(4):
                wtsp[q, pq, bq * 4 + kk] = wts[q, pids, kk]
            wtsa[q, pq, bq] = wts[q, pids, 0]
            # idx wrapped per chunk: position i in chunk -> [i%16, i//16]
            coff = pos0
            for ck in _chunks_for(caps[s]):
                li = np.arange(ck)
                v = loc[coff - pos0 : coff - pos0 + ck]
                idx16[q, li % 16, (coff + li * 0) // 16 + li // 16] = v.astype(
                    np.int16
                )
                coff += ck
        pos0 += caps[s]

    for rep in range(1, 8):
        idx16[:, rep * 16 : (rep + 1) * 16, :] = idx16[:, 0:16, :]
    return idx16, wtsp, wtsa, perm


# ---------------------------------------------------------------- device program

_NC_CACHE = {}


def build_program(caps):
    key = tuple(caps)
    if key in _NC_CACHE:
        return _NC_CACHE[key]
    NIT = sum(caps)
    NBT = NIT // 128
    f32 = mybir.dt.float32
    fb = BLEND_MB_DT
    nc = bass.Bass(num_swdge_queues=4)
    t4 = nc.declare_dram_parameter("t4", [R_TAB, 4 * C], TABLE_MB_DT, isOutput=False)
    idx_p = nc.declare_dram_parameter(
        "idx", [4, 128, NIT // 16], mybir.dt.int16, isOutput=False
    )
    wts_p = nc.declare_dram_parameter("wts", [4, 128, NBT * 4], fb, isOutput=False)
    wtsa_p = nc.declare_dram_parameter("wtsa", [4, 128, NBT], f32, isOutput=False)
    out_p = nc.declare_dram_parameter("out", [128, NBT * C], fb, isOutput=True)

    Copy = mybir.ActivationFunctionType.Copy
    add = mybir.AluOpType.add
    mult = mybir.AluOpType.mult
    amax = mybir.AluOpType.max

    # (seg, chunk_points, col16_offset, batch_offset) schedule
    sched = []
    col16 = 0
    batch = 0
    for s, cap in enumerate(caps):
        for ck in _chunks_for(cap):
            sched.append((s, ck, col16, batch))
            col16 += ck // 16
            batch += ck // 128

    with tile.TileContext(nc) as tc:
        with (
            tc.tile_pool(name="const", bufs=1) as cpool,
            tc.tile_pool(name="gp", bufs=8) as gpool,
            tc.tile_pool(name="tp", bufs=12) as tpool,
            tc.tile_pool(name="op", bufs=8) as opool,
            tc.tile_pool(name="mp", bufs=4) as mpool,
            tc.tile_pool(name="fp", bufs=3) as fpool,
        ):
            nc.gpsimd.load_library(mlp)
            _regs = {}

            def nreg(v):
                if v not in _regs:
                    r = nc.gpsimd.register(f"ni{v}").__enter__()
                    nc.gpsimd.reg_mov(r, v)
                    _regs[v] = r
                return _regs[v]

            idx_t = []
            wts_t = []
            wtsa_t = []
            for q in range(4):
                it = cpool.tile([128, NIT // 16], mybir.dt.int16, tag=f"idx{q}")
                wt = cpool.tile([128, NBT * 4], fb, tag=f"wts{q}")
                wa = cpool.tile([128, NBT], f32, tag=f"wtsa{q}")
                nc.sync.dma_start(out=it[:], in_=idx_p[q])
                nc.sync.dma_start(out=wt[:], in_=wts_p[q])
                nc.sync.dma_start(out=wa[:], in_=wtsa_p[q])
                idx_t.append(it)
                wts_t.append(wt)
                wtsa_t.append(wa)

            gi = [0]
            for s, ck, c16, b0 in sched:
                base, cnt = SEGS[s]
                nb = ck // 128
                otiles = []
                for q in range(4):
                    gt = gpool.tile([128, nb * 4 * C], TABLE_MB_DT, tag="g")
                    nc.gpsimd.dma_gather(
                        out_ap=gt[:].rearrange("p (n e) -> p n e", e=4 * C),
                        in_ap=t4[base : base + cnt],
                        idxs_ap=idx_t[q][:, c16 : c16 + ck // 16],
                        num_idxs=ck,
                        num_idxs_reg=nreg(ck),
                        elem_size=4 * C,
                        queue_num=gi[0] % 4,
                    )
                    gi[0] += 1
                    ot = opool.tile([128, nb * C], fb, tag="o")
                    for bi in range(nb):
                        bb = b0 + bi
                        wcol = bb * 4
                        wv = lambda kk: wts_t[q][:, wcol + kk : wcol + kk + 1]
                        ch = lambda kk: gt[
                            :, bi * 1024 + kk * 256 : bi * 1024 + (kk + 1) * 256
                        ]
                        t1 = tpool.tile([128, 256], fb, tag="t1")
                        t2 = tpool.tile([128, 256], fb, tag="t2")
                        t3 = tpool.tile([128, 256], fb, tag="t3")
                        # folded blend: o = WA*A + WB*B + WC*C + WD*D
                        nc.scalar.activation(
                            t1[:], ch(0), Copy,
                            scale=wtsa_t[q][:, bb : bb + 1],
                        )
                        nc.vector.scalar_tensor_tensor(
                            t2[:], ch(1), wv(1), t1[:], mult, add
                        )
                        nc.vector.scalar_tensor_tensor(
                            t3[:], ch(2), wv(2), t2[:], mult, add
                        )
                        nc.vector.scalar_tensor_tensor(
                            ot[:, bi * C : (bi + 1) * C],
                            ch(3), wv(3), t3[:], mult, add,
                        )
                    otiles.append(ot)

                m01 = mpool.tile([128, nb * C], fb, tag="m")
                m23 = mpool.tile([128, nb * C], fb, tag="m")
                fo = fpool.tile([128, nb * C], fb, tag="f")
                nc.vector.tensor_tensor(m01[:], otiles[0][:], otiles[1][:], amax)
                nc.vector.tensor_tensor(m23[:], otiles[2][:], otiles[3][:], amax)
                nc.vector.tensor_tensor(fo[:], m01[:], m23[:], amax)
                nc.sync.dma_start(
                    out=out_p[:, b0 * C : (b0 + nb) * C], in_=fo[:]
                )

    lower_extended_insts(nc)
    _NC_CACHE[key] = nc
    return nc


# ---------------------------------------------------------------- entry point


def kernel(p2, p3, p4, p5, rois, **run_kwargs):
    p2, p3, p4, p5, rois = (
        np.asarray(p2), np.asarray(p3), np.asarray(p4), np.asarray(p5),
        np.asarray(rois),
    )
    t4 = _build_t4(p2, p3, p4, p5)

    routed = []
    counts = np.zeros((N_CORES, len(SEGS)), np.int64)
    for core in range(N_CORES):
        chunk = rois[core * ROIS_PER_CORE : (core + 1) * ROIS_PER_CORE]
        seg_pids, rows, wts = _route_core(chunk)
        routed.append((seg_pids, rows, wts))
        counts[core] = [len(p) for p in seg_pids]

    caps = [int(-(-counts[:, s].max() // 128) * 128) for s in range(len(SEGS))]
    nc = build_program(caps)

    in_maps = []
    perms = []
    for core in range(N_CORES):
        seg_pids, rows, wts = routed[core]
        idx16, wtsp, wtsa, perm = _pack_core(seg_pids, rows, wts, caps)
        in_maps.append({"t4": t4, "idx": idx16, "wts": wtsp, "wtsa": wtsa})
        perms.append(perm)
    res = run_bass_kernel_spmd(nc, in_maps, core_ids=list(range(N_CORES)), **run_kwargs)

    NIT = sum(caps)
    NBT = NIT // 128
    outs = []
    for core in range(N_CORES):
        flat = (
            np.asarray(res.results[core]["out"])
            .astype(np.float32)
            .reshape(128, NBT, C)
            .transpose(1, 0, 2)
            .reshape(NIT, C)
        )
        perm = perms[core]
        pts = np.zeros((NPTS, C), np.float32)
        valid = perm >= 0
        pts[perm[valid]] = flat[valid]
        outs.append(
            pts.reshape(ROIS_PER_CORE, POOL, POOL, C).transpose(0, 3, 1, 2)
        )
    out = np.ascontiguousarray(np.concatenate(outs, axis=0))
    if run_kwargs:
        return out, res
    return out


# revision 26
# speedup vs baseline: 1.4235x; 1.1985x over previous
"""FPN ROI-align (crop + bilinear + 2x2 maxpool) Trainium2 kernel.

Problem: p2..p5 FPN pyramid [1,256,S,S] (S=256,128,64,32), rois [1000,5]
-> out [1000, 256, 7, 7] float32.

Strategy (8 NeuronCores, SPMD):
  - Shard ROIs: 125 per core; replicate the features.
  - Host builds a 4-corner NHWC table T4: row (lvl,y,x) =
    [T(y,x) | T(y+1c,x) | T(y,x+1c) | T(y+1c,x+1c)] (edge-clamped),
    1024 f16 per row.  One gathered row = all four bilinear corners of
    one sample: [A|B | C|D] (left column pair | right column pair).
  - Gathers use the Q7 ext-isa dma_gather (mlp library): ONE instruction
    fetches 512+ rows (int16 indices, ~1us fixed SWDGE cost amortized).
    int16 range forces segmenting T4 into four <=32768-row windows
    (p2 top half / p2 bottom half / p2 straddle window / p3+p4+p5);
    each output point is routed to a window that contains all four of
    its pool-plane sample rows.  Pad slots use idx 0 with zero weights.
  - Separable bilinear on-chip: xb = wx0*[A|B] + wx1*[C|D] (512-wide),
    o = wt*xb_top + wb*xb_bot (256-wide) via tensor_scalar +
    scalar_tensor_tensor (4x DVE perf mode on packed f16).  ACT engine
    takes the scalar multiplies for planes 0/1.
  - 2x2 maxpool = elementwise max across the four planes (chunk-wide).
  - Host un-permutes gather positions back to (roi, oy, ox) order.
"""
import os
import sys

import numpy as np

for _p in ("/opt/trn_rl_repo", "/root/.axon_site/_ro/trn_rl_repo"):
    if _p not in sys.path and os.path.isdir(_p):
        sys.path.append(_p)

import bass_rust  # noqa: E402
from concourse import bass, mybir  # noqa: E402
import concourse.tile as tile  # noqa: E402
from concourse.bass_utils import run_bass_kernel_spmd  # noqa: E402
from concourse.library_config import mlp  # noqa: E402
from concourse.library_overlay import lower_extended_insts  # noqa: E402
from concourse.vector_clock import ScopedClock  # noqa: E402

_MAX_WAITS = 1
_NOP_SEQ = [0]


def _patched_add_instruction(self, inst):
    """Wrap TileContext._add_instruction: the pinned walrus codegen allows
    at most one sync wait per instruction, so hoist excess waits onto
    single-wait NOPs queued just before on the same engine."""
    si = inst.sync_info
    if si is not None and len(si.on_wait) > _MAX_WAITS:
        waits = list(si.on_wait)
        extra, keep = waits[:-_MAX_WAITS], waits[-_MAX_WAITS:]
        for w in extra:
            _NOP_SEQ[0] += 1
            nop = bass_rust.InstNoOp(name=f"wsplit-{_NOP_SEQ[0]}", engine=inst.engine)
            nop.sync_info = bass_rust.SyncInfo(on_wait=[w], on_update=[])
            nop.bass_nofuse = True
            _orig_add_instruction(self, nop)
        inst.sync_info = bass_rust.SyncInfo(
            on_wait=keep, on_update=list(si.on_update)
        )
    _orig_add_instruction(self, inst)


_orig_add_instruction = tile.TileContext._add_instruction
if getattr(tile.TileContext, "_wsplit_patched", False):
    _orig_add_instruction = tile.TileContext._wsplit_orig
tile.TileContext._add_instruction = _patched_add_instruction
tile.TileContext._wsplit_patched = True
tile.TileContext._wsplit_orig = _orig_add_instruction


def _split_wait_drain_and_barrier(self, tick_clock, wait_clock):
    """Replacement for TileContext._drain_and_barrier (same wait limit)."""
    nc = self.nc
    probe = nc.sync.nop(nofuse=True)
    wait_clock.add_sem_waits(
        probe.ins, ScopedClock({None: tick_clock.global_clock})
    )
    si = probe.ins.sync_info
    waits = list(si.on_wait) if si is not None else []
    if si is not None:
        probe.ins.sync_info = bass_rust.SyncInfo(on_wait=waits[:1], on_update=[])
    for w in waits[1:]:
        n = nc.sync.nop(nofuse=True)
        n.ins.sync_info = bass_rust.SyncInfo(on_wait=[w], on_update=[])
    nc.sync.drain()

    nc.all_engine_barrier()
    assert self.sems is not None
    popped = nc._tile_sem_poison_stack.pop()
    assert popped is self._sem_poison
    nc.clear_and_free_semaphores(list(self.sems.allocated().values()))
    nc.all_engine_barrier()


tile.TileContext._drain_and_barrier = _split_wait_drain_and_barrier

# ---------------------------------------------------------------- constants
POOL = 7
PRE = 14
C = 256
N_ROIS = 1000
N_CORES = 8
ROIS_PER_CORE = N_ROIS // N_CORES          # 125
NPTS = ROIS_PER_CORE * POOL * POOL         # 6125 output points per core
LEVEL_HW = np.array([256, 128, 64, 32], np.int64)
BASES = np.array([0, 65536, 81920, 86016], np.int64)
R_TAB = 87040

# T4 row-index windows (start, row_count); every point's 4 sample rows
# must fit one window, indexed locally with int16 (< 32768).
SEGS = [
    (0, 32768),        # p2, y in [0, 128)
    (32768, 32768),    # p2, y in [128, 256)
    (16384, 32768),    # p2, y in [64, 192) -- boundary straddlers
    (65536, 21504),    # p3 + p4 + p5
]
CHUNK = 1024                               # points per dma_gather

import ml_dtypes  # noqa: E402

TABLE_NP_DT = ml_dtypes.bfloat16
TABLE_MB_DT = mybir.dt.bfloat16
BLEND_MB_DT = mybir.dt.bfloat16

# ---------------------------------------------------------------- host math


def _build_t4(p2, p3, p4, p5):
    """4-corner table [R_TAB, 1024]: row (lvl,y,x) =
    [T(y,x) | T(y+1c,x) | T(y,x+1c) | T(y+1c,x+1c)]."""
    parts = []
    for p in (p2, p3, p4, p5):
        L = np.transpose(p[0], (1, 2, 0)).astype(TABLE_NP_DT)  # [H, W, C]
        H, W = L.shape[0], L.shape[1]
        yb = np.minimum(np.arange(H) + 1, H - 1)
        xr = np.minimum(np.arange(W) + 1, W - 1)
        A = L
        B = L[yb]
        Cc = L[:, xr]
        D = L[yb][:, xr]
        parts.append(
            np.concatenate([A, B, Cc, D], axis=-1).reshape(-1, 4 * C)
        )
    return np.ascontiguousarray(np.concatenate(parts, axis=0))


def _roi_sample_data(rois):
    """f32-faithful mirror of the reference's coordinate math.

    Returns ilo [N,14,14] int64 (T4 row) and separable weights
    cw0, cw1 [N,14] (x, bounds-masked+edge-folded) and w_top, w_bot
    [N,14] (y, degenerate rows folded onto top)."""
    f32 = np.float32
    x1 = rois[:, 1].astype(f32)
    y1 = rois[:, 2].astype(f32)
    x2 = rois[:, 3].astype(f32)
    y2 = rois[:, 4].astype(f32)
    w = np.where(x2 - x1 <= 0, f32(1e-14), x2 - x1).astype(f32)
    h = np.where(y2 - y1 <= 0, f32(1e-14), y2 - y1).astype(f32)
    kf = f32(4.0) + np.log2(np.sqrt(w * h) / f32(224.0)).astype(f32)
    kf = np.clip(kf, f32(2.0), f32(5.0))
    k = np.round(kf)
    scale = np.exp2(k).astype(f32)
    lvl = k.astype(np.int64) - 2
    Wl = LEVEL_HW[lvl]
    x1s, y1s, x2s, y2s = x1 / scale, y1 / scale, x2 / scale, y2 / scale

    t = np.linspace(-1.0, 1.0, PRE, dtype=f32)
    px = (x1s + x2s)[:, None] * f32(0.5) + t[None, :] * ((x2s - x1s)[:, None] * f32(0.5))
    py = (y1s + y2s)[:, None] * f32(0.5) + t[None, :] * ((y2s - y1s)[:, None] * f32(0.5))

    u0 = np.floor(px)
    dx = (px - u0).astype(f32)
    u0i = u0.astype(np.int64)
    in_u0 = (u0i >= 0) & (u0i < Wl[:, None])
    in_u1 = (u0i + 1 >= 0) & (u0i + 1 < Wl[:, None])
    a0 = (f32(1.0) - dx) * in_u0
    a1 = dx * in_u1
    bx = np.clip(u0i, 0, (Wl - 2)[:, None])
    cw0 = a0 * (u0i == bx) + a1 * (u0i + 1 == bx)
    cw1 = a0 * (u0i == bx + 1) + a1 * (u0i + 1 == bx + 1)

    v0 = np.floor(py)
    dy = (py - v0).astype(f32)
    v0i = v0.astype(np.int64)
    b0 = (f32(1.0) - dy) * ((v0i >= 0) & (v0i < Wl[:, None]))
    b1 = dy * ((v0i + 1 >= 0) & (v0i + 1 < Wl[:, None]))
    ylo = np.clip(v0i, 0, (Wl - 1)[:, None])
    yhi = np.clip(v0i + 1, 0, (Wl - 1)[:, None])
    same = yhi == ylo
    w_top = b0 + b1 * same
    w_bot = b1 * (~same)

    base = BASES[lvl]
    ilo = base[:, None, None] + ylo[:, :, None] * Wl[:, None, None] + bx[:, None, :]
    return (
        ilo,
        cw0.astype(f32),
        cw1.astype(f32),
        w_top.astype(f32),
        w_bot.astype(f32),
    )


def _route_core(rois_chunk):
    """Segment-route one core's output points.

    Returns (seg_pids, rows, weights) where seg_pids[s] is the pid list
    (pid = roi*49 + oy*7 + ox) routed to segment s, rows [4, NPTS] is
    the global T4 row per plane, and weights [4, NPTS, 4] holds
    (wx0, wx1, wt, wb) per plane."""
    N = rois_chunk.shape[0]
    ilo, cw0, cw1, w_top, w_bot = _roi_sample_data(rois_chunk)

    oy, ox = np.meshgrid(np.arange(POOL), np.arange(POOL), indexing="ij")
    oyf = oy.reshape(-1)
    oxf = ox.reshape(-1)
    nn = np.repeat(np.arange(N), 49)
    oyr = np.tile(oyf, N)
    oxr = np.tile(oxf, N)

    rows = np.zeros((4, NPTS), np.int64)
    wts = np.zeros((4, NPTS, 4), np.float32)
    for q in range(4):
        a, b = q // 2, q % 2
        iy = 2 * oyr + a
        ix = 2 * oxr + b
        rows[q] = ilo[nn, iy, ix]
        # folded corner weights for chunks [A|B|C|D]
        wts[q, :, 0] = w_top[nn, iy] * cw0[nn, ix]
        wts[q, :, 1] = w_bot[nn, iy] * cw0[nn, ix]
        wts[q, :, 2] = w_top[nn, iy] * cw1[nn, ix]
        wts[q, :, 3] = w_bot[nn, iy] * cw1[nn, ix]

    rmin = rows.min(axis=0)
    rmax = rows.max(axis=0)
    seg_pids = []
    assigned = np.zeros(NPTS, bool)
    for s, (base, cnt) in enumerate(SEGS):
        ok = (~assigned) & (rmin >= base) & (rmax < base + cnt)
        seg_pids.append(np.nonzero(ok)[0])
        assigned |= ok
    assert assigned.all(), "point not routable to any T4 segment"
    return seg_pids, rows, wts


def _chunks_for(cap):
    """Split a segment capacity (multiple of 128) into dma_gather chunks."""
    out = []
    left = cap
    while left > 0:
        c = min(CHUNK, left)
        out.append(c)
        left -= c
    return out


def _pack_core(seg_pids, rows, wts, caps):
    """Pack one core's gather indices / weights / output permutation.

    Returns idx16 [4, 128, NIT//16] int16 (dma_gather wrapped layout,
    replicated across the 8 Q7 core stripes), wtsp [4, 128, NBT*4] f32,
    perm [NIT] int64 (pid per position, -1 for pad)."""
    NIT = sum(caps)
    NBT = NIT // 128
    idx16 = np.zeros((4, 128, NIT // 16), np.int16)
    wtsp = np.zeros((4, 128, NBT * 4), TABLE_NP_DT)
    wtsa = np.zeros((4, 128, NBT * 2), np.float32)
    perm = np.full(NIT, -1, np.int64)

    pos0 = 0
    for s, (base, cnt) in enumerate(SEGS):
        pids = seg_pids[s]
        n = len(pids)
        assert n <= caps[s]
        # positions pos0..pos0+caps[s]; chunk-wrapped idx layout
        perm[pos0 : pos0 + n] = pids
        for q in range(4):
            loc = np.zeros(caps[s], np.int64)
            loc[:n] = rows[q][pids] - base
            assert loc.max(initial=0) < cnt
            # weights land at (partition, batch) of the global position
            gpos = pos0 + np.arange(n)
            pq = gpos % 128
            bq = gpos // 128
            for kk in range(4):
                wtsp[q, pq, bq * 4 + kk] = wts[q, pids, kk]
            wtsa[q, pq, bq * 2] = wts[q, pids, 0]
            wtsa[q, pq, bq * 2 + 1] = wts[q, pids, 1]
            # idx wrapped per chunk: position i in chunk -> [i%16, i//16]
            coff = pos0
            for ck in _chunks_for(caps[s]):
                li = np.arange(ck)
                v = loc[coff - pos0 : coff - pos0 + ck]
                idx16[q, li % 16, (coff + li * 0) // 16 + li // 16] = v.astype(
                    np.int16
                )
                coff += ck
        pos0 += caps[s]

    for rep in range(1, 8):
        idx16[:, rep * 16 : (rep + 1) * 16, :] = idx16[:, 0:16, :]
    return idx16, wtsp, wtsa, perm


# ---------------------------------------------------------------- device program

_NC_CACHE = {}


def build_program(caps):
    key = tuple(caps)
    if key in _NC_CACHE:
        return _NC_CACHE[key]
    NIT = sum(caps)
    NBT = NIT // 128
    f32 = mybir.dt.float32
    fb = BLEND_MB_DT
    nc = bass.Bass(num_swdge_queues=4)
    t4 = nc.declare_dram_parameter("t4", [R_TAB, 4 * C], TABLE_MB_DT, isOutput=False)
    idx_p = nc.declare_dram_parameter(
        "idx", [4, 128, NIT // 16], mybir.dt.int16, isOutput=False
    )
    wts_p = nc.declare_dram_parameter("wts", [4, 128, NBT * 4], fb, isOutput=False)
    wtsa_p = nc.declare_dram_parameter("wtsa", [4, 128, NBT * 2], f32, isOutput=False)
    out_p = nc.declare_dram_parameter("out", [128, NBT * C], fb, isOutput=True)

    Copy = mybir.ActivationFunctionType.Copy
    add = mybir.AluOpType.add
    mult = mybir.AluOpType.mult
    amax = mybir.AluOpType.max

    # (seg, chunk_points, col16_offset, batch_offset) schedule
    sched = []
    col16 = 0
    batch = 0
    for s, cap in enumerate(caps):
        for ck in _chunks_for(cap):
            sched.append((s, ck, col16, batch))
            col16 += ck // 16
            batch += ck // 128

    with tile.TileContext(nc) as tc:
        with (
            tc.tile_pool(name="const", bufs=1) as cpool,
            tc.tile_pool(name="gp", bufs=6) as gpool,
            tc.tile_pool(name="tp", bufs=12) as tpool,
            tc.tile_pool(name="op", bufs=8) as opool,
            tc.tile_pool(name="mp", bufs=4) as mpool,
            tc.tile_pool(name="fp", bufs=3) as fpool,
        ):
            nc.gpsimd.load_library(mlp)
            _regs = {}

            def nreg(v):
                if v not in _regs:
                    r = nc.gpsimd.register(f"ni{v}").__enter__()
                    nc.gpsimd.reg_mov(r, v)
                    _regs[v] = r
                return _regs[v]

            idx_t = []
            wts_t = []
            wtsa_t = []
            for q in range(4):
                it = cpool.tile([128, NIT // 16], mybir.dt.int16, tag=f"idx{q}")
                wt = cpool.tile([128, NBT * 4], fb, tag=f"wts{q}")
                wa = cpool.tile([128, NBT * 2], f32, tag=f"wtsa{q}")
                nc.sync.dma_start(out=it[:], in_=idx_p[q])
                nc.sync.dma_start(out=wt[:], in_=wts_p[q])
                nc.sync.dma_start(out=wa[:], in_=wtsa_p[q])
                idx_t.append(it)
                wts_t.append(wt)
                wtsa_t.append(wa)

            gi = [0]
            for s, ck, c16, b0 in sched:
                base, cnt = SEGS[s]
                nb = ck // 128
                otiles = []
                for q in range(4):
                    gt = gpool.tile([128, nb * 4 * C], TABLE_MB_DT, tag="g")
                    nc.gpsimd.dma_gather(
                        out_ap=gt[:].rearrange("p (n e) -> p n e", e=4 * C),
                        in_ap=t4[base : base + cnt],
                        idxs_ap=idx_t[q][:, c16 : c16 + ck // 16],
                        num_idxs=ck,
                        num_idxs_reg=nreg(ck),
                        elem_size=4 * C,
                        queue_num=gi[0] % 4,
                    )
                    gi[0] += 1
                    ot = opool.tile([128, nb * C], fb, tag="o")
                    for bi in range(nb):
                        bb = b0 + bi
                        wcol = bb * 4
                        wv = lambda kk: wts_t[q][:, wcol + kk : wcol + kk + 1]
                        ch = lambda kk: gt[
                            :, bi * 1024 + kk * 256 : bi * 1024 + (kk + 1) * 256
                        ]
                        t1 = tpool.tile([128, 256], fb, tag="t1")
                        t1b = tpool.tile([128, 256], fb, tag="t1b")
                        t2 = tpool.tile([128, 256], fb, tag="t2")
                        t3 = tpool.tile([128, 256], fb, tag="t3")
                        # folded blend: o = WA*A + WB*B + WC*C + WD*D
                        # (ACT: the two scale-multiplies; DVE: add + 2 stt)
                        nc.scalar.activation(
                            t1[:], ch(0), Copy,
                            scale=wtsa_t[q][:, 2 * bb : 2 * bb + 1],
                        )
                        nc.scalar.activation(
                            t1b[:], ch(1), Copy,
                            scale=wtsa_t[q][:, 2 * bb + 1 : 2 * bb + 2],
                        )
                        nc.vector.tensor_tensor(t2[:], t1[:], t1b[:], add)
                        nc.vector.scalar_tensor_tensor(
                            t3[:], ch(2), wv(2), t2[:], mult, add
                        )
                        nc.vector.scalar_tensor_tensor(
                            ot[:, bi * C : (bi + 1) * C],
                            ch(3), wv(3), t3[:], mult, add,
                        )
                    otiles.append(ot)

                m01 = mpool.tile([128, nb * C], fb, tag="m")
                m23 = mpool.tile([128, nb * C], fb, tag="m")
                fo = fpool.tile([128, nb * C], fb, tag="f")
                nc.vector.tensor_tensor(m01[:], otiles[0][:], otiles[1][:], amax)
                nc.vector.tensor_tensor(m23[:], otiles[2][:], otiles[3][:], amax)
                nc.vector.tensor_tensor(fo[:], m01[:], m23[:], amax)
                nc.sync.dma_start(
                    out=out_p[:, b0 * C : (b0 + nb) * C], in_=fo[:]
                )

    lower_extended_insts(nc)
    _NC_CACHE[key] = nc
    return nc


# ---------------------------------------------------------------- entry point


def kernel(p2, p3, p4, p5, rois, **run_kwargs):
    p2, p3, p4, p5, rois = (
        np.asarray(p2), np.asarray(p3), np.asarray(p4), np.asarray(p5),
        np.asarray(rois),
    )
    t4 = _build_t4(p2, p3, p4, p5)

    routed = []
    counts = np.zeros((N_CORES, len(SEGS)), np.int64)
    for core in range(N_CORES):
        chunk = rois[core * ROIS_PER_CORE : (core + 1) * ROIS_PER_CORE]
        seg_pids, rows, wts = _route_core(chunk)
        routed.append((seg_pids, rows, wts))
        counts[core] = [len(p) for p in seg_pids]

    caps = [int(-(-counts[:, s].max() // 128) * 128) for s in range(len(SEGS))]
    nc = build_program(caps)

    in_maps = []
    perms = []
    for core in range(N_CORES):
        seg_pids, rows, wts = routed[core]
        idx16, wtsp, wtsa, perm = _pack_core(seg_pids, rows, wts, caps)
        in_maps.append({"t4": t4, "idx": idx16, "wts": wtsp, "wtsa": wtsa})
        perms.append(perm)
    res = run_bass_kernel_spmd(nc, in_maps, core_ids=list(range(N_CORES)), **run_kwargs)

    NIT = sum(caps)
    NBT = NIT // 128
    outs = []
    for core in range(N_CORES):
        flat = (
            np.asarray(res.results[core]["out"])
            .astype(np.float32)
            .reshape(128, NBT, C)
            .transpose(1, 0, 2)
            .reshape(NIT, C)
        )
        perm = perms[core]
        pts = np.zeros((NPTS, C), np.float32)
        valid = perm >= 0
        pts[perm[valid]] = flat[valid]
        outs.append(
            pts.reshape(ROIS_PER_CORE, POOL, POOL, C).transpose(0, 3, 1, 2)
        )
    out = np.ascontiguousarray(np.concatenate(outs, axis=0))
    if run_kwargs:
        return out, res
    return out
